# revision 1
# baseline (speedup 1.0000x reference)
"""TRN2 Bass/Tile kernel for nn_ClassifierHetero (batched heterograph classifier).

In the reference forward, the HeteroGraphConv stack is dead code (its outputs
are deleted and never read): the module output depends only on the per-graph
means of the ORIGINAL node features, concatenated to [B, 4], followed by a
3-layer MLP -> [B, 10].

Sharding (per the hint): data-parallel over graphs — 8 graphs per core x 8
cores; the tiny MLP weights are replicated.

Device program (v4):
  - ALL device data is bf16 (device rel-err tolerance is 2e-2; bf16 rounding
    contributes ~5e-3). Halves DMA bytes and makes every PE matmul
    single-pass (no fp32 LOW/HIGH double pumping).
  - The host pre-scales node features by 1/max(count,1) of their graph, so
    free-dim sums ARE the per-graph partial means.
  - One [128, WD] bf16 buffer per core, columns ordered ports-first:
    [p0 | p1 | comp | net | Wc2 | Wc3 | sel]. It is DMA'd in TWO waves of
    column-chunks (wave A = ports, wave B = rest), each wave split across
    the three rings (SP + ACT HWDGE, gpsimd SWDGE) as full-128-partition
    transfers (row-chunked transfers degenerate to ONE DMA engine; column
    chunks spread across all 16 and run at ~200GB/s per HWDGE ring). The
    port reduce then overlaps wave B's transfer. The tiny W1/bias buffer
    goes FIRST on the SP ring. Wave A + W1 increment semA (gate >= 64),
    wave B increments semB (gate >= 48).
  - comp/net share one width so their two sums are a single 3D-AP
    TENSOR_REDUCE ([128,2,W] -> [128,2]); port0/port1 likewise.
  - Biases are folded into the PE: bc1 rides as a 5th row of W1 against a
    constant 1/16 column in S2 (the selector matmul then yields an exact
    1.0 row in hgT); bc2/bc3 prefill their PSUM banks via K=1 matmuls
    against a ones row while the reduces run. The DVE activations are pure
    relu-with-immediate and the final bias-add is a plain PSUM copy.
  - Raw (non-tile) buffer reads are gated per ENGINE: each of DVE/PE gets
    its own wait_ge instruction per semaphore (a cross-engine dep edge with
    sync=False emits NO semaphore and races!). Everything downstream is
    ordered by engine program order + the tile scheduler's own cross-engine
    sems, keeping every instruction at <= ONE sync-wait (the
    bass2jax/neuronxcc codegen limit).
  - The TileContext tail is re-emitted as a chain of single-wait drains and
    ends with one sem-only barrier (no semaphore clear: NRT zeroes
    semaphores at execution start).

Self-contained: all shapes/constants hardcoded from the problem spec.
"""

import numpy as np

try:
    import ml_dtypes

    BF16 = ml_dtypes.bfloat16
except ImportError:  # pragma: no cover
    BF16 = None

# --- problem constants (hardcoded from the spec) ---
B = 64            # graphs in the batch
NCORES = 8
G = B // NCORES   # graphs per core
HID = 128
NCLS = 10
NSUB = 16         # SBUF partitions per graph: partition p = g*NSUB + s
P_FULL = G * NSUB  # = 128

# Default per-graph column widths (capacity per graph = NSUB * W). comp and
# net share one width so their two sums are a single 3D-AP reduce; ports
# likewise. Graph sizes are ~Binomial(N, 1/64); defaults cover >5 sigma and
# auto-escalate (with recompile) if an input ever exceeds them.
W_CN0, W_P0 = 160, 416

# Qt layout: [5, QW] bf16. cols 0:HID = W1 rows (comp, net, p0, p1 order)
# with row 4 = bc1; row 0 of cols HID:2*HID = bc2; row 0 of the next NCLS
# cols = bc3; row 0 of the last G cols = ones.
_QB2 = HID
_QB3 = _QB2 + HID
_QON = _QB3 + NCLS
QW = _QON + G

_NC_CACHE: dict = {}


def _round_up(x: int, m: int) -> int:
    return -(-x // m) * m


def _widths(cnt_c, cnt_p, cnt_n):
    def w_for(maxcnt, w0):
        need = _round_up(_round_up(int(maxcnt), NSUB) // NSUB, 16)
        return max(w0, need)

    return (
        w_for(max(cnt_c.max(), cnt_n.max()), W_CN0),
        w_for(cnt_p.max(), W_P0),
    )


def _offsets(wcn: int, wp: int):
    """Column layout: [p0 | p1 | comp | net | Wc2 | Wc3 | sel]."""
    off_c = 2 * wp
    off_w2 = 2 * wp + 2 * wcn
    off_w3 = off_w2 + HID
    off_sel = off_w3 + NCLS
    wd = _round_up(off_sel + G, 8)
    return off_c, off_w2, off_w3, off_sel, wd


def _wave_chunks(wcn: int, wp: int):
    """Two waves of column chunks: wave A = ports (cols [0, 2*wp)), wave B =
    the rest. Each wave splits across (sync, scalar, gpsimd). The SWDGE ring
    is slower (~115 GB/s vs ~200) and the sync ring also carries the W1/bias
    buffer + wave B first, so scalar gets the largest share of wave A."""
    wa = 2 * wp
    _, _, _, _, wd = _offsets(wcn, wp)
    s1 = _round_up(wa * 46 // 100, 8)
    # wave A: two chunks on the HWDGE rings (sync's ring runs a bit slower
    # in practice, so it gets less); wave B: one SWDGE transfer
    return [(0, s1), (s1, wa)], [(wa, wd)]


def _patch_tile_tail():
    """The neuronxcc codegen used by the bass2jax path allows only ONE
    sync-wait command per instruction, but TileContext's kernel-tail drain
    waits on every live semaphore at once. Re-emit that tail as a chain of
    single-wait drains (one per logical processor of the global clock),
    then halt after a single sem-only barrier — no semaphore clearing
    (NRT zeroes semaphores at execution start)."""
    import concourse.tile as tile

    if getattr(tile.TileContext, "_single_wait_tail", False):
        return
    from concourse.vector_clock import ScopedClock, VectorClock

    def _drain_and_barrier(self, tick_clock, wait_clock):
        nc = self.nc
        gc = tick_clock.global_clock
        n = len(gc)
        for proc in range(n):
            t = gc[proc]
            if t <= 0:
                continue
            sub = VectorClock([0] * n)
            sub.require_at_least(proc, t)
            d = nc.sync.drain(fusable=False)
            wait_clock.add_sem_waits(d.ins, ScopedClock({None: sub}))
        nc.sync.drain(fusable=False)
        nc.all_engine_barrier(sem_only=True)
        assert self.sems is not None
        popped = nc._tile_sem_poison_stack.pop()
        assert popped is self._sem_poison

    tile.TileContext._drain_and_barrier = _drain_and_barrier
    tile.TileContext._single_wait_tail = True


def _build_nc(wcn: int, wp: int):
    import concourse.bass as bass
    import concourse.mybir as mybir
    import concourse.tile as tile
    from concourse.tile import add_dep_helper

    _patch_tile_tail()
    f32 = mybir.dt.float32
    bf16 = mybir.dt.bfloat16
    X = mybir.AxisListType.X
    MAX = mybir.AluOpType.max
    off_c, off_w2, off_w3, off_sel, wd = _offsets(wcn, wp)
    wave_a, wave_b = _wave_chunks(wcn, wp)

    nc = bass.Bass()

    da_ext = [
        nc.declare_dram_parameter(f"da{i}", [P_FULL, c1 - c0], bf16, isOutput=False)
        for i, (c0, c1) in enumerate(wave_a)
    ]
    db_ext = [
        nc.declare_dram_parameter(f"db{i}", [P_FULL, c1 - c0], bf16, isOutput=False)
        for i, (c0, c1) in enumerate(wave_b)
    ]
    q_ext = nc.declare_dram_parameter("qw1", [5, QW], bf16, isOutput=False)
    out_ext = nc.declare_dram_parameter("out", [NCLS, G], f32, isOutput=True)

    Dt = nc.alloc_sbuf_tensor("Dt", [P_FULL, wd], bf16)
    Qt = nc.alloc_sbuf_tensor("Qt", [5, QW], bf16)
    semA = nc.alloc_semaphore("dma_a")
    semB = nc.alloc_semaphore("dma_b")

    with nc.Block(no_gpsimd_drain=True) as blk:

        @blk.sync
        def _(s):
            c0, c1 = wave_a[0]
            s.dma_start(out=Dt[:, c0:c1], in_=da_ext[0][:]).then_inc(semA, 16)

        @blk.scalar
        def _(s):
            c0, c1 = wave_a[1]
            s.dma_start(out=Dt[:, c0:c1], in_=da_ext[1][:]).then_inc(semA, 16)
            s.dma_start(out=Qt[:], in_=q_ext[:]).then_inc(semA, 16)

        @blk.gpsimd
        def _(s):
            c0, c1 = wave_b[0]
            s.dma_start(out=Dt[:, c0:c1], in_=db_ext[0][:]).then_inc(semB, 16)

    gates = []

    def gate(engine, sem, val):
        # emitted with wait value 0 so the Tile scheduling sim (which never
        # executes the pre-block's increments) doesn't deadlock; the real
        # value is patched post-schedule.
        g = engine.wait_ge(sem, 0)
        gates.append((g, val))
        return g

    with tile.TileContext(nc) as tc:
        with (
            tc.tile_pool(name="sbuf", bufs=1) as pool,
            tc.tile_pool(name="psum", bufs=1, space="PSUM") as psum,
        ):
            S = pool.tile([P_FULL, 4], f32)     # reduce accumulators (f32)
            S2 = pool.tile([P_FULL, 5], bf16)   # bf16 copy + const 1/16 col
            hgT = pool.tile([5, G], bf16)
            h1 = pool.tile([HID, G], bf16)
            h2 = pool.tile([HID, G], bf16)
            otT = pool.tile([NCLS, G], f32)
            ps_hg = psum.tile([5, G], f32)
            ps_h1 = psum.tile([HID, G], f32)
            ps_h2 = psum.tile([HID, G], f32)
            ps_oT = psum.tile([NCLS, G], f32)

            dep = []  # (raw-buffer reader, same-engine gate) edges

            # constant 1/16 column (DVE, no data deps; emitted before any
            # gate so it executes during the transfers). All S2 writers stay
            # on DVE so the selector matmul needs only ONE (DVE) wait.
            nc.vector.memset(S2[:, 4:5], 1.0 / NSUB)

            # --- DVE reduces; the port one overlaps wave B's transfer ----
            # S cols: 0=comp, 1=net, 2=p0, 3=p1
            gA_v = gate(nc.vector, semA, 48)
            p_view = Dt[:, 0 : 2 * wp].rearrange("p (t w) -> p t w", t=2)
            r = nc.vector.reduce_sum(S[:, 2:4], p_view, axis=X)
            dep.append((r, gA_v))
            gB_v = gate(nc.vector, semB, 16)
            cn_view = Dt[:, off_c : off_c + 2 * wcn].rearrange(
                "p (t w) -> p t w", t=2
            )
            r = nc.vector.reduce_sum(S[:, 0:2], cn_view, axis=X)
            dep.append((r, gB_v))
            nc.vector.tensor_copy(S2[:, 0:4], S[:])

            # --- PE: bias prefills under the transfers -------------------
            gA_t = gate(nc.tensor, semA, 48)
            r = nc.tensor.matmul(
                ps_h2[:], lhsT=Qt[0:1, _QB2 : _QB2 + HID],
                rhs=Qt[0:1, _QON : _QON + G], start=True, stop=False,
            )
            dep.append((r, gA_t))
            r = nc.tensor.matmul(
                ps_oT[:], lhsT=Qt[0:1, _QB3 : _QB3 + NCLS],
                rhs=Qt[0:1, _QON : _QON + G], start=True, stop=False,
            )
            dep.append((r, gA_t))
            gB_t = gate(nc.tensor, semB, 16)

            # collapse 16 scaled partials per graph -> means [5, G]
            # (row 4 = sum of 1/16 over each graph's 16 partitions = 1.0)
            r = nc.tensor.matmul(
                ps_hg[:], lhsT=S2[:], rhs=Dt[:, off_sel : off_sel + G],
                start=True, stop=True,
            )
            dep.append((r, gB_t))
            nc.vector.tensor_copy(hgT[:], ps_hg[:])

            # layer 1: h1T = relu(W1aug.T @ hgT)  (bc1 = W1aug row 4)
            r = nc.tensor.matmul(
                ps_h1[:], lhsT=Qt[0:5, 0:HID], rhs=hgT[:], start=True, stop=True
            )
            dep.append((r, gA_t))
            nc.vector.tensor_scalar(h1[:], ps_h1[:], 0.0, None, op0=MAX)
            # layer 2: h2T = relu(Wc2.T @ h1T + bc2)  (bc2 prefilled)
            r = nc.tensor.matmul(
                ps_h2[:], lhsT=Dt[:, off_w2 : off_w2 + HID], rhs=h1[:],
                start=False, stop=True,
            )
            dep.append((r, gB_t))
            nc.vector.tensor_scalar(h2[:], ps_h2[:], 0.0, None, op0=MAX)
            # layer 3 (transposed): outT = Wc3.T @ h2T + bc3  [NCLS, G]
            r = nc.tensor.matmul(
                ps_oT[:], lhsT=Dt[:, off_w3 : off_w3 + NCLS], rhs=h2[:],
                start=False, stop=True,
            )
            dep.append((r, gB_t))
            nc.vector.tensor_copy(otT[:], ps_oT[:])
            nc.sync.dma_start(out=out_ext[:], in_=otT[:])

            for consumer, g in dep:
                add_dep_helper(
                    consumer.ins, g.ins, False, "raw input read after DMA gate"
                )

    for g, val in gates:
        g.ins.sync_info.on_wait[0].wait_value = val

    # the bass2jax/neuronxcc codegen rejects >1 sync-wait per instruction —
    # fail fast at build time instead of deep inside the compiler
    for f in nc.m.functions:
        for blk in f.blocks:
            for ins in blk.instructions:
                si = getattr(ins, "sync_info", None)
                if si is not None and si.on_wait and len(si.on_wait) > 1:
                    raise AssertionError(
                        f"{type(ins).__name__} {ins.name} has "
                        f"{len(si.on_wait)} sync waits"
                    )
    return nc


def _get_nc(wcn: int, wp: int):
    key = (wcn, wp)
    if key not in _NC_CACHE:
        _NC_CACHE[key] = _build_nc(wcn, wp)
    return _NC_CACHE[key]


def _pack_col(out, col_off, h, col, bounds, width, scale):
    """Pack one (node type, feature col) into out[:, :, col_off:col_off+width],
    scaling graph b's values by scale[b] (zero-padded to NSUB*width)."""
    cap = NSUB * width
    for b in range(B):
        m, g = divmod(b, G)
        s, e = int(bounds[b]), int(bounds[b + 1])
        n = e - s
        if n == 0:
            continue
        buf = np.zeros(cap, np.float32)
        buf[:n] = h[s:e, col] * scale[b]
        p0 = g * NSUB
        out[m, p0 : p0 + NSUB, col_off : col_off + width] = (
            buf.reshape(NSUB, width)
        )


def _prepare(inputs):
    h_comp = np.ascontiguousarray(np.asarray(inputs["h_comp"], dtype=np.float32))
    h_port = np.ascontiguousarray(np.asarray(inputs["h_port"], dtype=np.float32))
    h_net = np.ascontiguousarray(np.asarray(inputs["h_net"], dtype=np.float32))
    gid_c = np.asarray(inputs["gid_comp"])
    gid_p = np.asarray(inputs["gid_port"])
    gid_n = np.asarray(inputs["gid_net"])

    edges = np.arange(B + 1)
    bc = np.searchsorted(gid_c, edges)
    bp = np.searchsorted(gid_p, edges)
    bn = np.searchsorted(gid_n, edges)
    cnt_c = np.diff(bc)
    cnt_p = np.diff(bp)
    cnt_n = np.diff(bn)

    wcn, wp = _widths(cnt_c, cnt_p, cnt_n)
    off_c, off_w2, off_w3, off_sel, wd = _offsets(wcn, wp)

    Wc1 = np.asarray(inputs["Wc1"], dtype=np.float32)
    bc1 = np.asarray(inputs["bc1"], dtype=np.float32)
    Wc2 = np.asarray(inputs["Wc2"], dtype=np.float32)
    bc2 = np.asarray(inputs["bc2"], dtype=np.float32)
    Wc3 = np.asarray(inputs["Wc3"], dtype=np.float32)
    bc3 = np.asarray(inputs["bc3"], dtype=np.float32)

    rc = 1.0 / np.maximum(cnt_c, 1)
    rp = 1.0 / np.maximum(cnt_p, 1)
    rn = 1.0 / np.maximum(cnt_n, 1)

    sel = (np.arange(P_FULL)[:, None] // NSUB == np.arange(G)[None, :]).astype(
        np.float32
    )

    D = np.zeros((NCORES, P_FULL, wd), np.float32)
    _pack_col(D, 0, h_port, 0, bp, wp, rp)
    _pack_col(D, wp, h_port, 1, bp, wp, rp)
    _pack_col(D, off_c, h_comp, 0, bc, wcn, rc)
    _pack_col(D, off_c + wcn, h_net, 0, bn, wcn, rn)
    D[:, :, off_w2 : off_w2 + HID] = Wc2
    D[:, :, off_w3 : off_w3 + NCLS] = Wc3
    D[:, :, off_sel : off_sel + G] = sel
    Db = D.astype(BF16)
    wave_a, wave_b = _wave_chunks(wcn, wp)
    Da = [np.ascontiguousarray(Db[:, :, c0:c1]) for c0, c1 in wave_a]
    Dbw = [np.ascontiguousarray(Db[:, :, c0:c1]) for c0, c1 in wave_b]

    # device mean order is (comp, net, p0, p1); reference hg column order is
    # (comp, p0, p1, net) -> permute W1 rows to match, append bc1 as row 4
    Q = np.zeros((5, QW), np.float32)
    Q[0:4, 0:HID] = Wc1[[0, 3, 1, 2], :]
    Q[4, 0:HID] = bc1
    Q[0, _QB2 : _QB2 + HID] = bc2
    Q[0, _QB3 : _QB3 + NCLS] = bc3
    Q[0, _QON : _QON + G] = 1.0
    Qb = np.ascontiguousarray(Q.astype(BF16))

    in_maps = [
        {"da0": Da[0][m], "da1": Da[1][m], "db0": Dbw[0][m],
         "qw1": Qb, "_full": Db[m]}
        for m in range(NCORES)
    ]
    return (wcn, wp), in_maps


def _run(inputs, trace=False, **kwargs):
    from concourse.bass_utils import run_bass_kernel_spmd

    (wcn, wp), in_maps = _prepare(inputs)
    in_maps = [{k: v for k, v in im.items() if k != "_full"} for im in in_maps]
    nc = _get_nc(wcn, wp)
    res = run_bass_kernel_spmd(
        nc, in_maps, list(range(NCORES)), trace=trace, **kwargs
    )
    # per-core output is [NCLS, G] (classes on partitions) — transpose back
    out = np.concatenate(
        [res.results[m]["out"].T for m in range(NCORES)], axis=0
    ).astype(np.float32)
    return out, res


def kernel(**inputs) -> np.ndarray:
    out, _ = _run(inputs, trace=False)
    return out


def run_traced(inputs, **kwargs):
    out, res = _run(inputs, trace=True, **kwargs)
    return out, res


def simulate_numpy(**inputs):
    """Numpy emulation of the device program (for fast logic validation)."""
    (wcn, wp), in_maps = _prepare(inputs)
    off_c, off_w2, off_w3, off_sel, wd = _offsets(wcn, wp)
    outs = []
    for m in range(NCORES):
        D = in_maps[m]["_full"].astype(np.float32)
        Q = in_maps[m]["qw1"].astype(np.float32)
        S = np.zeros((P_FULL, 5), np.float32)
        S[:, 0] = D[:, off_c : off_c + wcn].sum(1)
        S[:, 1] = D[:, off_c + wcn : off_c + 2 * wcn].sum(1)
        S[:, 2] = D[:, 0:wp].sum(1)
        S[:, 3] = D[:, wp : 2 * wp].sum(1)
        S2 = S.astype(BF16).astype(np.float32)
        S2[:, 4] = 1.0 / NSUB
        sel = D[:, off_sel : off_sel + G]
        hgT = (S2.T @ sel).astype(BF16).astype(np.float32)  # [5, G]
        h1 = np.maximum(Q[0:5, 0:HID].T @ hgT, 0.0)
        h1 = h1.astype(BF16).astype(np.float32)
        bias2 = Q[0:1, _QB2 : _QB2 + HID].T @ Q[0:1, _QON : _QON + G]
        h2 = np.maximum(D[:, off_w2 : off_w2 + HID].T @ h1 + bias2, 0.0)
        h2 = h2.astype(BF16).astype(np.float32)
        bias3 = Q[0:1, _QB3 : _QB3 + NCLS].T @ Q[0:1, _QON : _QON + G]
        oT = D[:, off_w3 : off_w3 + NCLS].T @ h2 + bias3
        outs.append(oT.T)
    return np.concatenate(outs, 0).astype(np.float32)

